# revision 1
# baseline (speedup 1.0000x reference)
"""AffinityBasedAveraging Trainium2 kernel — fp16, wide-tile, DMA-fold version.

Same math as the baseline (softmax over 9 offsets, weighted average of
shifted embeddings) but all device tensors are float16:
  - DVE tensor_tensor ops hit the 2x_1p perf mode (2 results/cycle) for
    operands that are 16-bit, stride-1, 4B-aligned. The ox=0 taps read E
    at an odd element offset (2B) and stay at 1x — unavoidable parity.
  - All DMA traffic (aff in, emb in, out, folds) halves.
Host pre-casts inputs to f16 and upcasts the f16 output back to f32;
expected rel err ~1e-3 vs the 2e-2 gate.

Sharding: 8 cores = 4 batches x 2 H-halves (as baseline).
"""

import numpy as np

import bass_rust
import concourse.bass as bass
import concourse.mybir as mybir
import concourse.tile as tile
from concourse.bass_utils import run_bass_kernel_spmd

F16 = mybir.dt.float16
F32 = mybir.dt.float32
AF = mybir.ActivationFunctionType
OP = mybir.AluOpType
AX = mybir.AxisListType

B, C, H, W = 4, 16, 512, 512
K = 9
OFFSETS = [(-1, -1), (-1, 0), (-1, 1), (0, -1), (0, 0), (0, 1), (1, -1), (1, 0), (1, 1)]
N_CORES = 8
HH = H // 2
YT = 128

_wsplit_ctr = [0]


def _split_multi_waits(nc):
    """Walrus here rejects >1 semaphore wait per instruction; split extras
    into same-engine NoOp prefixes."""
    n = 0
    for f in nc.m.functions:
        for bb in f.blocks:
            insts = bb.instructions
            if not any(
                i.sync_info is not None and len(i.sync_info.on_wait or []) > 1
                for i in insts
            ):
                continue
            new = []
            for inst in insts:
                si = inst.sync_info
                waits = list(si.on_wait) if si is not None and si.on_wait else []
                if len(waits) > 1:
                    for w in waits[:-1]:
                        _wsplit_ctr[0] += 1
                        nop = mybir.InstNoOp(name=f"I-wsplit-{_wsplit_ctr[0]}")
                        nop.engine = inst.engine
                        nop.sync_info = bass_rust.SyncInfo(on_wait=[w], on_update=[])
                        new.append(nop)
                        n += 1
                    inst.sync_info = bass_rust.SyncInfo(
                        on_wait=[waits[-1]], on_update=list(si.on_update or [])
                    )
                new.append(inst)
            insts[:] = new
    return n


def build_nc(
    split_waits=True,
    reps=1,
    dma_folds=3,
    xc=256,
    xe=512,
    e_shift_mode="hbm3x",
    tree_sum=True,
    ebufs=1,
    tmpbufs=2,
    accbufs=2,
    xbufs=2,
    sbufs=1,
    chains=1,
    hw_loop=False,
    sink_out=False,
):
    import contextlib

    nc = bass.Bass("TRN2", target_bir_lowering=False, debug=False, num_devices=N_CORES)
    aff = nc.declare_dram_parameter("aff", [K, HH, W], F16, isOutput=False)
    emb = nc.declare_dram_parameter("emb", [C, HH + 2, W + 2], F16, isOutput=False)
    if sink_out:
        out = nc.declare_dram_parameter("out", [C, HH, W], F16, isOutput=False)
        tok = nc.declare_dram_parameter("tok", [128, 16], F32, isOutput=True)
    else:
        out = nc.declare_dram_parameter("out", [C, HH, W], F16, isOutput=True)

    with tile.TileContext(nc) as tc:
        with (
            nc.allow_low_precision(reason="f16 kernel; tolerance gate is 2e-2"),
            tc.tile_pool(name="p_a", bufs=2) as p_a,
            tc.tile_pool(name="p_x", bufs=xbufs) as p_x,
            tc.tile_pool(name="p_e", bufs=ebufs) as p_e,
            tc.tile_pool(name="p_acc", bufs=accbufs) as p_acc,
            tc.tile_pool(name="p_tmp", bufs=tmpbufs) as p_tmp,
            tc.tile_pool(name="p_s", bufs=sbufs) as p_s,
        ):
            if sink_out:
                tokt = p_s.tile([128, 16], F32, tag="tok")
                nc.vector.memset(tokt[:], 1.0)
                nc.sync.dma_start(out=tok[:, :], in_=tokt[:])
            rep_iter = (
                tc.For_i(0, reps) if hw_loop else contextlib.nullcontext(range(reps))
            )
            with rep_iter as _it:
             for _rep in ([0] if hw_loop else range(reps)):
              for ty in range(HH // YT):
                ys = ty * YT
                A = p_a.tile([YT, K, W], F16, tag="A")
                nc.sync.dma_start(
                    out=A[:],
                    in_=aff[:, ys : ys + YT, :].rearrange("k y x -> y k x"),
                )
                X = p_x.tile([YT, K, W], F16, tag="X")
                nc.scalar.activation(X[:], A[:], AF.Exp)
                t4 = p_s.tile([YT, 4, W], F16, tag="t4")
                nc.vector.tensor_tensor(t4[:], X[:, 0:4, :], X[:, 4:8, :], OP.add)
                t2 = p_s.tile([YT, 2, W], F16, tag="t2")
                nc.vector.tensor_tensor(t2[:], t4[:, 0:2, :], t4[:, 2:4, :], OP.add)
                t1 = p_s.tile([YT, 1, W], F16, tag="t1")
                nc.vector.tensor_tensor(t1[:], t2[:, 0:1, :], t2[:, 1:2, :], OP.add)
                S = p_s.tile([YT, 1, W], F16, tag="S")
                nc.vector.tensor_tensor(S[:], t1[:], X[:, 8:9, :], OP.add)
                R = p_s.tile([YT, 1, W], F16, tag="R")
                nc.vector.reciprocal(R[:], S[:])
                # in-place normalize: X becomes W (same-index elementwise,
                # safe on the streaming DVE; frees the separate W allocation)
                Wfull = X
                nc.vector.tensor_tensor(
                    Wfull[:], X[:], R[:, 0:1, :].to_broadcast((YT, K, W)), OP.mult
                )
                for xh in range(W // xc):
                    xs = xh * xc
                    Wt = Wfull[:, :, xs : xs + xc]

                    if e_shift_mode == "hbm3x":
                        xew = xc if xe is None else xe
                        if xh % (xew // xc) == 0:
                            e_tiles = {}
                            xes = (xs // xew) * xew
                            for oy in (-1, 0, 1):
                                t = p_e.tile([YT, C, xew + 2], F16, tag=f"E{oy}")
                                rs = ys + oy + 1
                                nc.sync.dma_start(
                                    out=t[:],
                                    in_=emb[
                                        :, rs : rs + YT, xes : xes + xew + 2
                                    ].rearrange("c y x -> y c x"),
                                )
                                e_tiles[oy] = t
                            build_nc._e_tiles = e_tiles
                        eoff = xs % xew
                        E = {
                            oy: build_nc._e_tiles[oy][:, :, eoff : eoff + xc + 2]
                            for oy in (-1, 0, 1)
                        }
                    else:
                        t0 = p_e.tile([YT, C, xc + 2], F16, tag="E0")
                        nc.sync.dma_start(
                            out=t0[:],
                            in_=emb[:, ys + 1 : ys + 1 + YT, xs : xs + xc + 2].rearrange(
                                "c y x -> y c x"
                            ),
                        )
                        tm = p_e.tile([YT, C, xc + 2], F16, tag="E-1")
                        nc.sync.dma_start(out=tm[1:YT], in_=t0[0 : YT - 1])
                        nc.sync.dma_start(
                            out=tm[0:1],
                            in_=emb[:, ys : ys + 1, xs : xs + xc + 2].rearrange(
                                "c y x -> y c x"
                            ),
                        )
                        tp = p_e.tile([YT, C, xc + 2], F16, tag="E1")
                        nc.sync.dma_start(out=tp[0 : YT - 1], in_=t0[1:YT])
                        nc.sync.dma_start(
                            out=tp[YT - 1 : YT],
                            in_=emb[
                                :, ys + YT + 1 : ys + YT + 2, xs : xs + xc + 2
                            ].rearrange("c y x -> y c x"),
                        )
                        E = {-1: tm, 0: t0, 1: tp}

                    acc = p_acc.tile([YT, C, xc], F16, tag="acc")
                    tmp = p_tmp.tile([YT, C, xc], F16, tag="tmp")

                    def tap(k):
                        oy, ox = OFFSETS[k]
                        wk = Wt[:, k, :][:, None, :].to_broadcast((YT, C, xc))
                        return wk, E[oy][:, :, 1 + ox : 1 + ox + xc]

                    n_dve_taps = K - (dma_folds + 1 if dma_folds else 0)
                    if dma_folds:
                        folds = []
                        for j in range(dma_folds + 1):
                            ft = p_acc.tile([YT, C, xc], F16, tag=f"fold{j}")
                            wk, ek = tap(n_dve_taps + j)
                            nc.vector.tensor_tensor(ft[:], wk, ek, OP.mult)
                            folds.append(ft)
                        for j in range(1, dma_folds + 1):
                            for xq in range(0, xc, 128):
                                nc.gpsimd.dma_start(
                                    out=folds[0][:, :, xq : xq + 128],
                                    in_=folds[j][:, :, xq : xq + 128],
                                    accum_op=OP.add,
                                )
                    if chains == 2 and n_dve_taps >= 4:
                        acc1 = p_acc.tile([YT, C, xc], F16, tag="acc1")
                        tmp1 = p_tmp.tile([YT, C, xc], F16, tag="tmp1")
                        ka = [k for k in range(n_dve_taps) if k % 2 == 0]
                        kb = [k for k in range(n_dve_taps) if k % 2 == 1]
                        # interleave two independent mult/add chains
                        for i in range(max(len(ka), len(kb))):
                            for ks, a_t, t_t in ((ka, acc, tmp), (kb, acc1, tmp1)):
                                if i >= len(ks):
                                    continue
                                wk, ek = tap(ks[i])
                                if i == 0:
                                    nc.vector.tensor_tensor(a_t[:], wk, ek, OP.mult)
                                else:
                                    nc.vector.tensor_tensor(t_t[:], wk, ek, OP.mult)
                                    nc.vector.tensor_tensor(
                                        a_t[:], a_t[:], t_t[:], OP.add
                                    )
                        nc.vector.tensor_tensor(acc[:], acc[:], acc1[:], OP.add)
                    else:
                        for k in range(n_dve_taps):
                            wk, ek = tap(k)
                            if k == 0:
                                nc.vector.tensor_tensor(acc[:], wk, ek, OP.mult)
                            else:
                                nc.vector.tensor_tensor(tmp[:], wk, ek, OP.mult)
                                nc.vector.tensor_tensor(acc[:], acc[:], tmp[:], OP.add)
                    if dma_folds:
                        for xq in range(0, xc, 128):
                            nc.gpsimd.dma_start(
                                out=acc[:, :, xq : xq + 128],
                                in_=folds[0][:, :, xq : xq + 128],
                                accum_op=OP.add,
                            )

                    nc.sync.dma_start(
                        out=out[:, ys : ys + YT, xs : xs + xc].rearrange(
                            "c y x -> y c x"
                        ),
                        in_=acc[:],
                    )

    if split_waits:
        _split_multi_waits(nc)
    return nc


_nc_cache = None


def _get_nc():
    global _nc_cache
    if _nc_cache is None:
        _nc_cache = build_nc()
    return _nc_cache


def shard_inputs(affinities, embedding):
    """Full f32 inputs -> 8 per-core f16 input maps (batch x H-half)."""
    affinities = np.asarray(affinities)
    embedding = np.asarray(embedding)
    ycl = lambda idx: np.clip(idx, 0, H - 1)
    xcl = np.clip(np.arange(-1, W + 1), 0, W - 1)
    in_maps = []
    for i in range(N_CORES):
        b, half = i // 2, i % 2
        y0 = half * HH
        aff_s = np.ascontiguousarray(
            affinities[b, :, y0 : y0 + HH, :].astype(np.float16)
        )
        rows = ycl(np.arange(y0 - 1, y0 + HH + 1))
        emb_s = np.ascontiguousarray(embedding[b][:, rows][:, :, xcl].astype(np.float16))
        in_maps.append({"aff": aff_s, "emb": emb_s})
    return in_maps


def unshard_outputs(results):
    out = np.empty((B, C, H, W), np.float32)
    for i in range(N_CORES):
        b, half = i // 2, i % 2
        y0 = half * HH
        out[b, :, y0 : y0 + HH, :] = results[i]["out"].astype(np.float32)
    return out


def kernel(affinities, embedding):
    nc = _get_nc()
    in_maps = shard_inputs(affinities, embedding)
    try:
        res = run_bass_kernel_spmd(nc, in_maps, list(range(N_CORES)))
    except Exception:
        import time as _t

        _t.sleep(2.0)
        res = run_bass_kernel_spmd(nc, in_maps, list(range(N_CORES)))
    out = unshard_outputs(res.results)
    kernel.last_result = res
    return out



# revision 10
# speedup vs baseline: 1.0614x; 1.0614x over previous
"""AffinityBasedAveraging Trainium2 kernel — PE-accumulate version.

Math: softmax over 9 offset affinities, weighted average of 3x3-shifted
embeddings (replicate-padded). All device tensors f16; host casts.

Engine split (per core = 1 batch x 1 H-half of 256 rows, 2 y-tiles of 128):
  - Act   : exp(aff); odd-parity shifted weight copy (W9o); PSUM->SBUF evict
  - DVE   : softmax tree-sum + reciprocal + normalize; 7 of 9 weighted
            products  rhs_k = w_k (bcast over C) * E_shift   (all f16,
            stride-1, even byte offsets -> 2x_1p dual-pump mode)
  - Pool  : the other 2 weighted products
  - PE    : the 3x3 accumulation: for each tap k an identity-matmul
            accumulates rhs_k into PSUM f32 (start on k==0, stop on k==8).
            The x-shift per tap is applied by reading rhs at an AP column
            offset delta = 1+ox; the y-shift comes from the three
            row-shifted E tiles loaded straight from HBM (hbm3x).
  - DMA   : aff in, 3x emb in (row-shifted), out f16

Weight parity trick: products must keep every DVE operand at even byte
offsets to hold the 2x_1p mode. W9pad stores weights at column offset 2,
W9o (Act copy) at offset 1; tap ox in {-1,0,+1} picks the slice so both
the W and E operands always start at even element offsets, and the PE
read offset delta fixes up the remaining shift.

Sharding: 8 cores = 4 batches x 2 H-halves (same as baseline).
"""

import numpy as np

import bass_rust
import concourse.bass as bass
import concourse.mybir as mybir
import concourse.tile as tile
from concourse.bass_utils import run_bass_kernel_spmd
from concourse.masks import make_identity

F16 = mybir.dt.float16
F32 = mybir.dt.float32
AF = mybir.ActivationFunctionType
OP = mybir.AluOpType

B, C, H, W = 4, 16, 512, 512
K = 9
OFFSETS = [(-1, -1), (-1, 0), (-1, 1), (0, -1), (0, 0), (0, 1), (1, -1), (1, 0), (1, 1)]
N_CORES = 8
HH = H // 2
YT = 128
XH = 256  # x half width

_wsplit_ctr = [0]


def _split_multi_waits(nc):
    """Walrus rejects >1 semaphore wait per instruction; split extras into
    same-engine NoOp prefixes."""
    n = 0
    # Fallback stationary AP for PE wait-carrier Ldweights (identity matrix).
    fallback_wap = None
    for f in nc.m.functions:
        for bb in f.blocks:
            for i in bb.instructions:
                if isinstance(i, mybir.InstMatmult):
                    fallback_wap = i.ins[1]
                    break
            if fallback_wap is not None:
                break
        if fallback_wap is not None:
            break
    for f in nc.m.functions:
        for bb in f.blocks:
            insts = bb.instructions
            # PE hwdecode can't run NoOp: convert framework-emitted PE NoOps
            # into redundant Ldweights of the identity.
            for idx, i in enumerate(insts):
                if isinstance(i, mybir.InstNoOp) and i.engine == mybir.EngineType.PE:
                    assert fallback_wap is not None
                    repl = mybir.InstLdweights(
                        name=i.name, ins=[fallback_wap], outs=[]
                    )
                    repl.engine = mybir.EngineType.PE
                    repl.sync_info = i.sync_info
                    insts[idx] = repl
            if not any(
                i.sync_info is not None and len(i.sync_info.on_wait or []) > 1
                for i in insts
            ):
                continue
            new = []
            for inst in insts:
                si = inst.sync_info
                waits = list(si.on_wait) if si is not None and si.on_wait else []
                if len(waits) > 1:
                    for w in waits[:-1]:
                        _wsplit_ctr[0] += 1
                        if inst.engine == mybir.EngineType.PE:
                            # PE hwdecode can't run NoOp: carry the wait on a
                            # redundant Ldweights of the same stationary AP
                            # (the following Matmult reloads weights anyway).
                            if isinstance(inst, mybir.InstMatmult):
                                wap = inst.ins[1]
                            elif isinstance(inst, mybir.InstLdweights):
                                wap = inst.ins[0]
                            elif fallback_wap is not None:
                                wap = fallback_wap
                            else:
                                raise AssertionError(
                                    f"PE multi-wait on {type(inst).__name__}"
                                )
                            nop = mybir.InstLdweights(
                                name=f"I-wsplit-{_wsplit_ctr[0]}",
                                ins=[wap],
                                outs=[],
                            )
                        else:
                            nop = mybir.InstNoOp(name=f"I-wsplit-{_wsplit_ctr[0]}")
                        nop.engine = inst.engine
                        nop.sync_info = bass_rust.SyncInfo(on_wait=[w], on_update=[])
                        new.append(nop)
                        n += 1
                    inst.sync_info = bass_rust.SyncInfo(
                        on_wait=[waits[-1]], on_update=list(si.on_update or [])
                    )
                new.append(inst)
            insts[:] = new
    return n


def build_nc(
    split_waits=True,
    reps=1,
    hw_loop=False,
    sink_out=False,
    dma_folds=None,  # accepted for test.py back-compat; unused (no SWDGE here)
    pool_taps=(5, 6),
    xq=128,  # x chunk width (must divide W, even)
    last_chunk_no_pool=True,  # final chunk of final tile: all taps on DVE
    out_dma_per_chunk=True,
):
    import contextlib

    nc = bass.Bass("TRN2", target_bir_lowering=False, debug=False, num_devices=N_CORES)
    aff = nc.declare_dram_parameter("aff", [K, HH, W], F16, isOutput=False)
    emb = nc.declare_dram_parameter("emb", [C, HH + 2, W + 2], F16, isOutput=False)
    if sink_out:
        out = nc.declare_dram_parameter("out", [C, HH, W], F16, isOutput=False)
        tok = nc.declare_dram_parameter("tok", [128, 16], F32, isOutput=True)
    else:
        out = nc.declare_dram_parameter("out", [C, HH, W], F16, isOutput=True)

    with tile.TileContext(nc) as tc:
        with (
            nc.allow_low_precision(reason="f16 kernel; tolerance gate is 2e-2"),
            tc.tile_pool(name="p_a", bufs=2) as p_a,
            tc.tile_pool(name="p_e", bufs=1) as p_e,
            tc.tile_pool(name="p_w", bufs=1) as p_w,
            tc.tile_pool(name="p_t", bufs=1) as p_t,
            tc.tile_pool(name="p_r", bufs=1) as p_r,
            tc.tile_pool(name="p_o", bufs=1) as p_o,
            tc.tile_pool(name="p_c", bufs=1) as p_c,
            tc.tile_pool(name="psum", bufs=1, space="PSUM") as p_ps,
        ):
            if sink_out:
                tokt = p_c.tile([128, 16], F32, tag="tok")
                nc.vector.memset(tokt[:], 1.0)
                nc.sync.dma_start(out=tok[:, :], in_=tokt[:])

            ident = p_c.tile([128, 128], F16, tag="ident")

            rep_iter = (
                tc.For_i(0, reps) if hw_loop else contextlib.nullcontext(range(reps))
            )
            with rep_iter as _it:
             for _rep in ([0] if hw_loop else range(reps)):
              make_identity(nc, ident[:])
              for ty in range(HH // YT):
                ys = ty * YT
                A = p_a.tile([YT, K, W], F16, tag="A")
                nc.sync.dma_start(
                    out=A[:],
                    in_=aff[:, ys : ys + YT, :].rearrange("k y x -> y k x"),
                )
                E = {}
                for oy in (-1, 0, 1):
                    t = p_e.tile([YT, C, W + 2], F16, tag=f"E{oy}")
                    rs = ys + oy + 1
                    nc.sync.dma_start(
                        out=t[:],
                        in_=emb[:, rs : rs + YT, 0 : W + 2].rearrange(
                            "c y x -> y c x"
                        ),
                    )
                    E[oy] = t

                X = p_w.tile([YT, K, W], F16, tag="X")
                nc.scalar.activation(X[:], A[:], AF.Exp)
                t4 = p_t.tile([YT, 4, W], F16, tag="t4")
                nc.vector.tensor_tensor(t4[:], X[:, 0:4, :], X[:, 4:8, :], OP.add)
                t2 = p_t.tile([YT, 2, W], F16, tag="t2")
                nc.vector.tensor_tensor(t2[:], t4[:, 0:2, :], t4[:, 2:4, :], OP.add)
                t1 = p_t.tile([YT, 1, W], F16, tag="t1")
                nc.vector.tensor_tensor(t1[:], t2[:, 0:1, :], t2[:, 1:2, :], OP.add)
                S = p_t.tile([YT, 1, W], F16, tag="S")
                nc.vector.tensor_tensor(S[:], t1[:], X[:, 8:9, :], OP.add)
                R = p_t.tile([YT, 1, W], F16, tag="R")
                nc.vector.reciprocal(R[:], S[:])

                # Normalized weights at column offset 2 (even parity) and a
                # shifted odd-parity copy at offset 1 (Act engine).
                W9 = p_w.tile([YT, K, W + 4], F16, tag="W9")
                nc.vector.memset(W9[:, :, 0:2], 0.0)
                nc.vector.memset(W9[:, :, W + 2 : W + 4], 0.0)
                nc.vector.tensor_tensor(
                    W9[:, :, 2 : W + 2],
                    X[:],
                    R[:, 0:1, :].to_broadcast((YT, K, W)),
                    OP.mult,
                )
                W9o = p_w.tile([YT, K, W + 4], F16, tag="W9o")
                nc.vector.memset(W9o[:, :, 0:1], 0.0)
                nc.vector.memset(W9o[:, :, W + 1 : W + 4], 0.0)
                nc.scalar.copy(W9o[:, :, 1 : W + 1], W9[:, :, 2 : W + 2])

                out_sb = p_o.tile([YT, C, W], F16, tag="out_sb")

                n_chunks = W // xq
                # psum chunk: [YT, cpc, xq] f32 must be one bank (512 f32)
                cpc = max(1, 512 // xq)  # channels per matmul pass
                n_cc = C // 2 // cpc  # passes per half-psum tile
                for h in range(n_chunks):
                    x0 = h * xq
                    is_last = ty == HH // YT - 1 and h == n_chunks - 1
                    pool_set = () if (last_chunk_no_pool and is_last) else pool_taps
                    # DVE taps first, pool taps last: PE consumes in this
                    # order so the slow Pool products arrive in time.
                    tap_order = [k for k in range(K) if k not in pool_set] + [
                        k for k in range(K) if k in pool_set
                    ]
                    rhs = {}
                    for k in tap_order:
                        oy, ox = OFFSETS[k]
                        s = 1 + ox
                        rk = p_r.tile([YT, C, xq + 2], F16, tag=f"r{k}", name=f"r{k}")
                        if s == 1:
                            wop = W9o[:, k, x0 : x0 + xq + 2]
                        elif s == 0:
                            wop = W9[:, k, x0 + 2 : x0 + xq + 4]
                        else:  # s == 2
                            wop = W9[:, k, x0 : x0 + xq + 2]
                        wb = wop[:, None, :].to_broadcast((YT, C, xq + 2))
                        eop = E[oy][:, :, x0 : x0 + xq + 2]
                        eng = nc.gpsimd if k in pool_set else nc.vector
                        eng.tensor_tensor(rk[:], wb, eop, OP.mult)
                        rhs[k] = rk

                    ps = [
                        p_ps.tile([YT, C // 2, xq], F32, tag=f"ps{chh}", name=f"ps{chh}")
                        for chh in range(2)
                    ]
                    for pos, k in enumerate(tap_order):
                        oy, ox = OFFSETS[k]
                        d = 1 + ox
                        for chh in range(2):
                            for cc in range(n_cc):
                                c0 = chh * (C // 2) + cpc * cc
                                nc.tensor.matmul(
                                    ps[chh][:, cpc * cc : cpc * cc + cpc, :],
                                    ident[:],
                                    rhs[k][:, c0 : c0 + cpc, d : d + xq],
                                    start=(pos == 0),
                                    stop=(pos == K - 1),
                                )
                    for chh in range(2):
                        nc.scalar.copy(
                            out_sb[:, chh * (C // 2) : (chh + 1) * (C // 2), x0 : x0 + xq],
                            ps[chh][:],
                        )
                    if out_dma_per_chunk:
                        nc.sync.dma_start(
                            out=out[:, ys : ys + YT, x0 : x0 + xq].rearrange(
                                "c y x -> y c x"
                            ),
                            in_=out_sb[:, :, x0 : x0 + xq],
                        )

                if not out_dma_per_chunk:
                    nc.sync.dma_start(
                        out=out[:, ys : ys + YT, :].rearrange("c y x -> y c x"),
                        in_=out_sb[:],
                    )

    if split_waits:
        _split_multi_waits(nc)
    return nc


_nc_cache = None


def _get_nc():
    global _nc_cache
    if _nc_cache is None:
        _nc_cache = build_nc()
    return _nc_cache


def shard_inputs(affinities, embedding):
    """Full f32 inputs -> 8 per-core f16 input maps (batch x H-half)."""
    affinities = np.asarray(affinities)
    embedding = np.asarray(embedding)
    ycl = lambda idx: np.clip(idx, 0, H - 1)
    xcl = np.clip(np.arange(-1, W + 1), 0, W - 1)
    in_maps = []
    for i in range(N_CORES):
        b, half = i // 2, i % 2
        y0 = half * HH
        aff_s = np.ascontiguousarray(
            affinities[b, :, y0 : y0 + HH, :].astype(np.float16)
        )
        rows = ycl(np.arange(y0 - 1, y0 + HH + 1))
        emb_s = np.ascontiguousarray(embedding[b][:, rows][:, :, xcl].astype(np.float16))
        in_maps.append({"aff": aff_s, "emb": emb_s})
    return in_maps


def unshard_outputs(results):
    out = np.empty((B, C, H, W), np.float32)
    for i in range(N_CORES):
        b, half = i // 2, i % 2
        y0 = half * HH
        out[b, :, y0 : y0 + HH, :] = results[i]["out"].astype(np.float32)
    return out


def kernel(affinities, embedding):
    nc = _get_nc()
    in_maps = shard_inputs(affinities, embedding)
    try:
        res = run_bass_kernel_spmd(nc, in_maps, list(range(N_CORES)))
    except Exception:
        import time as _t

        _t.sleep(2.0)
        res = run_bass_kernel_spmd(nc, in_maps, list(range(N_CORES)))
    out = unshard_outputs(res.results)
    kernel.last_result = res
    return out
